# revision 1
# baseline (speedup 1.0000x reference)
"""Trainium2 Bass kernel for CustomSoftmaxExperts (topk_masking).

Math: reference computes softmax over the 64-expert axis, finds the 5th
largest softmax value per row, and keeps values >= max(kth, 0.2).
Since softmax rows sum to 1, at most 4 values can be >= 0.2, so any value
>= 0.2 is automatically within the top-5: the mask reduces EXACTLY to
``softmax >= 0.2`` (verified bit-identical against the jax reference).

Kernel per row (64 contiguous f32 in DRAM):
    e = exp(x)            # no max-subtract needed: |x| <= ~5.5, exp <= ~250
    s = sum(e); r = 1/s
    soft = e * r
    out  = (soft >= 0.2) ? soft : 0

Sharding: 32*8192 = 262144 rows, data-parallel over 8 cores ->
32768 rows/core (8.39 MB in + 8.39 MB out per core; memory-bound,
per-core HBM roofline ~358 GB/s -> ~47 us).

Layout per core: flat [32768*64] viewed as NTILES x [128 partitions x FREE],
FREE = 2048 (32 rows of 64 per partition line; 8 KB/partition DMA lines).
Engines: ACT does exp; DVE does segmented reduce_sum [128,K,64]->[128,K],
reciprocal, broadcast multiply, and a fused scalar_tensor_tensor
(soft >= 0.2) * soft.
"""

import numpy as np

import concourse.bacc as bacc
import concourse.mybir as mybir
from concourse import bass_utils
from concourse.tile import TileContext

N_CORES = 8
ROWS_TOTAL = 32 * 8192
E = 64  # experts per row
ROWS_PER_CORE = ROWS_TOTAL // N_CORES  # 32768
P = 128  # SBUF partitions
FREE = 2048  # f32 elements per partition line per tile
K = FREE // E  # rows per partition line
TILE_ROWS = P * K  # rows per tile
NTILES = ROWS_PER_CORE // TILE_ROWS
THRESHOLD = 0.2

_cached = None


TOT_FD = ROWS_PER_CORE * E // P  # 16384 f32 per partition
# graded tile schedule: small tiles at the ends for fast pipeline fill/drain
GRADED = (512, 512, 1024, 2048, 2048, 2048, 2048, 2048, 2048, 1024, 512, 512)
VARIANT = "dve"  # "dve" | "poolmul" | "pooltail"


def _build(hw_reps: int = 0, variant: str | None = None, bufs: int = 3,
           fds=GRADED, pool_frac: float = 0.0):
    """Build the per-core program. hw_reps>0 wraps the body in a hardware
    For_i loop that re-runs it hw_reps times (for on-device timing only).
    pool_frac: fraction of the mul/stt columns offloaded to GPSIMD."""
    variant = VARIANT if variant is None else variant
    assert sum(fds) == TOT_FD
    f32 = mybir.dt.float32
    nc = bacc.Bacc(
        "TRN2",
        target_bir_lowering=False,
        debug=False,
        num_devices=N_CORES,
    )
    x_d = nc.dram_tensor("x", [ROWS_PER_CORE * E], f32, kind="ExternalInput")
    o_d = nc.dram_tensor("o", [ROWS_PER_CORE * E], f32, kind="ExternalOutput")
    x_f = x_d.ap().rearrange("(p f) -> p f", p=P)
    o_f = o_d.ap().rearrange("(p f) -> p f", p=P)

    with TileContext(nc) as tc:
        with tc.tile_pool(name="work", bufs=bufs) as pool:

            def body():
                off = 0
                for fd in fds:
                    K = fd // E
                    xt = pool.tile([P, fd], f32, tag="x", name="xt")
                    nc.sync.dma_start(xt[:], x_f[:, off:off + fd])
                    et = pool.tile([P, fd], f32, tag="e", name="et")
                    nc.scalar.activation(
                        et[:], xt[:], mybir.ActivationFunctionType.Exp
                    )
                    e3 = et[:].rearrange("p (k c) -> p k c", c=E)
                    st = pool.tile([P, K], f32, tag="s", name="st")
                    nc.vector.reduce_sum(st[:], e3, axis=mybir.AxisListType.X)
                    rt = pool.tile([P, K], f32, tag="r", name="rt")
                    nc.vector.reciprocal(rt[:], st[:])
                    softt = pool.tile([P, fd], f32, tag="soft", name="softt")
                    s3 = softt[:].rearrange("p (k c) -> p k c", c=E)
                    ot = pool.tile([P, fd], f32, tag="o", name="ot")
                    # column split: normalize of rows [0, kd) runs on gpsimd
                    # (TT mult is Pool-valid); mask+apply stays on DVE
                    kd = int(K * pool_frac + 0.5)
                    if kd > 0:
                        nc.gpsimd.tensor_mul(
                            s3[:, 0:kd],
                            e3[:, 0:kd],
                            rt[:, 0:kd].broadcast_to([P, kd, E]),
                        )
                    if kd < K:
                        nc.vector.tensor_mul(
                            s3[:, kd:K],
                            e3[:, kd:K],
                            rt[:, kd:K].broadcast_to([P, K - kd, E]),
                        )
                    nc.vector.scalar_tensor_tensor(
                        ot[:],
                        softt[:],
                        THRESHOLD,
                        softt[:],
                        op0=mybir.AluOpType.is_ge,
                        op1=mybir.AluOpType.mult,
                    )
                    nc.sync.dma_start(o_f[:, off:off + fd], ot[:])
                    off += fd

            if hw_reps > 0:
                with tc.For_i(0, hw_reps, 1):
                    body()
            else:
                body()
    nc.compile()
    return nc


def kernel(inputs: np.ndarray) -> np.ndarray:
    global _cached
    if _cached is None:
        _cached = _build()
    nc = _cached

    x = np.ascontiguousarray(inputs, dtype=np.float32).reshape(N_CORES, -1)
    in_maps = [{"x": x[c]} for c in range(N_CORES)]
    res = bass_utils.run_bass_kernel_spmd(nc, in_maps, core_ids=list(range(N_CORES)))
    out = np.concatenate([res.results[c]["o"] for c in range(N_CORES)])
    return out.reshape(inputs.shape).astype(np.float32, copy=False)

